# revision 11
# baseline (speedup 1.0000x reference)
"""CenterLoss Trainium2 kernel.

reference semantics:
    feats  = features.reshape(4096, 96)
    label  = argmax(predicts.reshape(4096, 6625), axis=1)   # first occurrence
    d[n]   = ||feats[n] - centers[label[n]]||^2
    loss   = (sum_n clip(d[n], 1e-12, 1e12) + (4096*6625-4096)*1e-12) / 4096

Sharding: data-parallel over the flattened 4096-row batch axis, 512 rows per
core across 8 cores; centers replicated. Each core returns 128 per-partition
distance sums; the host does the final (tiny) reduction ("all-reduce").

Per-core pipeline (phases batched over the 4 row-tiles of 128 rows each so
engines never stall on each other's in-order queues; partition p holds rows
4p..4p+3 so the features DMA is one contiguous 1536B line per partition):
  A. stream predicts tiles [128, 6625] HBM->SBUF in two half-tile DMAs
     (~1.7 MB each, 13.3KB/partition descriptors; the memory-bound part,
     ~13.6 MB/core) + one features DMA
  B. per-half max-reduce over [128, chunks, 53] views -> cmax [128, 4*125];
     per-tile DVE max8 -> row max top8, max_index -> first-occurrence
     chunk id (uint32)
  C. indirect-DMA gather of each row's winning 53-wide chunk (212B/row,
     ~108KB/core extra HBM vs 543KB for 265-wide chunks); max8+max_index on
     the gathered chunk -> position; label = 53*chunk + pos; indirect-DMA
     gather of centers[label]
  D. acc[p] = sum over tiles/dims of (f - c)^2 via one gpsimd subtract +
     one ACT Square with accumulate
max8/max_index tie-breaking is first-occurrence, matching jnp.argmax
bit-exactly (f32 compares are exact).
"""

import numpy as np

NUM_CLASSES = 6625
FEAT_DIM = 96
N_ROWS = 4096           # B*T = 64*64
N_CORES = 8
ROWS_PER_CORE = N_ROWS // N_CORES   # 512
P = 128                 # partitions
N_TILES = ROWS_PER_CORE // P        # 4 row-tiles per core
CH = 53                 # chunk size (6625 = 125 * 53)
NCHUNK = NUM_CLASSES // CH          # 125
OUT_COLS = 1            # per-partition distance sums

_CACHE = {}
_DEFAULT_VER = 3


def _splits(n, k):
    """k split sizes of n, near-equal, larger first."""
    q, r = divmod(n, k)
    return [q + 1] * r + [q] * (k - r)


def _build_nc(reps=1, ver=None, **kw):
    """Dispatch: ver=3 (default) = fp16 pair-tree kernel; ver=0 = baseline."""
    ver = _DEFAULT_VER if ver is None else ver
    if ver == 3:
        return _build_nc_v3(reps=reps, **kw)
    return _build_nc_base(reps=reps, **kw)


def _build_nc_base(reps=1, ablate="full", nsplit=2, rsplit=None, dual=0,
                   dsub=0, ilv=1, xbufs=4, gat1=0, seng=0, pipe=0):
    if rsplit is None:
        rsplit = nsplit
    assert nsplit % rsplit == 0
    key = ("nc", reps, ablate, nsplit, rsplit, dual, dsub, ilv, xbufs, gat1,
           seng, pipe)
    if key in _CACHE:
        return _CACHE[key]

    from contextlib import ExitStack

    import concourse.bass as bass
    import concourse.tile as tile
    from concourse import bacc, mybir

    nc = bacc.Bacc(
        "TRN2",
        target_bir_lowering=False,
        debug=False,
        num_devices=N_CORES,
    )

    predicts = nc.dram_tensor(
        "predicts", [ROWS_PER_CORE, NUM_CLASSES], mybir.dt.float32,
        kind="ExternalInput",
    )
    features = nc.dram_tensor(
        "features", [ROWS_PER_CORE, FEAT_DIM], mybir.dt.float32,
        kind="ExternalInput",
    )
    centers = nc.dram_tensor(
        "centers", [NUM_CLASSES, FEAT_DIM], mybir.dt.float32,
        kind="ExternalInput",
    )
    out = nc.dram_tensor(
        "out", [P, OUT_COLS], mybir.dt.float32, kind="ExternalOutput",
    )

    fadd = mybir.AluOpType.add
    fmul = mybir.AluOpType.mult

    with tile.TileContext(nc) as tc:
        with ExitStack() as ctx:
            xpool = ctx.enter_context(tc.tile_pool(name="x", bufs=xbufs))
            small = ctx.enter_context(tc.tile_pool(name="small", bufs=5))
            const = ctx.enter_context(tc.tile_pool(name="const", bufs=1))

            # prebase[p, t] = rowid(p, t) * 125 : global chunk-row id base
            # rowid = t*128 + p normally; 4p + t with interleaved mapping
            pb4_i = const.tile([P, N_TILES], mybir.dt.int32)
            if ilv:
                nc.gpsimd.iota(
                    pb4_i[:], pattern=[[NCHUNK, N_TILES]], base=0,
                    channel_multiplier=N_TILES * NCHUNK)
            else:
                nc.gpsimd.iota(
                    pb4_i[:], pattern=[[P * NCHUNK, N_TILES]], base=0,
                    channel_multiplier=NCHUNK)
            pb4 = const.tile([P, N_TILES], mybir.dt.float32)
            nc.vector.tensor_copy(pb4[:], pb4_i[:])

            # per-partition distance accumulator (summed over row tiles)
            acc = const.tile([P, 1], mybir.dt.float32)

            # warm the ACT Square table set while DMAs stream
            actwarm = const.tile([P, 1], mybir.dt.float32)
            nc.scalar.activation(
                actwarm[:], pb4[:, 0:1],
                mybir.ActivationFunctionType.Square)

            # predicts viewed as rows of 53 elements: [512*125, 53]
            pred_chunks = predicts.ap().rearrange("r (a b) -> (r a) b", b=CH)
            if ilv:
                # partition p holds rows 4p..4p+3; tile t = rows {4p+t}
                pred_v = predicts.ap().rearrange(
                    "(p t) c -> p t c", t=N_TILES)
                feat_v = features.ap().rearrange(
                    "(p t) d -> p t d", t=N_TILES)
            else:
                pred_v = None
                feat_v = features.ap().rearrange("(t p) d -> p t d", p=P)

            # chunk-count split per partial DMA; reduces cover rsplit
            # groups of nsplit//rsplit DMA splits each
            QSPLIT = _splits(NCHUNK, nsplit)
            QOFF = [0]
            for q in QSPLIT:
                QOFF.append(QOFF[-1] + q)
            step = nsplit // rsplit
            ROFF = [QOFF[i * step] for i in range(rsplit)] + [NCHUNK]

            for _ in range(reps):
                # ---- phase A: stream predicts + features ----
                xs = []
                for t in range(N_TILES):
                    x = xpool.tile([P, NUM_CLASSES], mybir.dt.float32, tag="x")
                    for q in range(nsplit):
                        c0, c1 = QOFF[q] * CH, QOFF[q + 1] * CH
                        if seng:
                            eng = nc.gpsimd
                        else:
                            eng = nc.scalar if (dual and q % 2) else nc.sync
                        if ilv:
                            src = pred_v[:, t:t + 1, c0:c1]
                        else:
                            src = predicts.ap()[t * P:(t + 1) * P, c0:c1]
                        eng.dma_start(x[:, c0:c1], src)
                    xs.append(x)
                ftile = small.tile(
                    [P, N_TILES * FEAT_DIM], mybir.dt.float32, tag="feat")
                nc.sync.dma_start(ftile[:], feat_v)

                if ablate == "dma":
                    for t in range(N_TILES):
                        xv = xs[t][:].rearrange("p (a b) -> p a b", b=CH)
                        nc.vector.tensor_reduce(
                            acc[:, 0:1], xv[:, 0:1, :],
                            axis=mybir.AxisListType.XY, op=mybir.AluOpType.max)
                        nc.vector.tensor_reduce(
                            acc[:, 0:1], xv[:, NCHUNK - 1:NCHUNK, :],
                            axis=mybir.AxisListType.XY, op=mybir.AluOpType.max)
                    continue

                # ---- phase B: chunk maxes + per-tile top8/argmax chunk ----
                # pipe mode: each tile's L1 argmax + chunk gather issue right
                # after that tile's reduces, so the SWDGE gather latency
                # hides under the next tile's reduce work instead of
                # stalling the in-order DVE queue before phase C.
                cmax4 = small.tile(
                    [P, N_TILES * NCHUNK], mybir.dt.float32, tag="cmax4")
                top8 = small.tile(
                    [P, N_TILES * 8], mybir.dt.float32, tag="top8")
                cidx8 = small.tile(
                    [P, N_TILES * 8], mybir.dt.uint32, tag="cidx8")
                cidx_f = small.tile([P, N_TILES], mybir.dt.float32,
                                    tag="cidx_f")
                rsi4 = small.tile([P, N_TILES], mybir.dt.int32, tag="rsi4")
                chunkcat = small.tile(
                    [P, N_TILES * CH], mybir.dt.float32, tag="chunkcat")
                cidx8v = cidx8[:].rearrange("p (t e) -> p t e", e=8)

                def l1_tile(t):
                    nc.vector.max(
                        top8[:, t * 8:(t + 1) * 8],
                        cmax4[:, t * NCHUNK:(t + 1) * NCHUNK])
                    nc.vector.max_index(
                        cidx8[:, t * 8:(t + 1) * 8],
                        top8[:, t * 8:(t + 1) * 8],
                        cmax4[:, t * NCHUNK:(t + 1) * NCHUNK])
                    nc.vector.tensor_copy(
                        cidx_f[:, t:t + 1], cidx8v[:, t:t + 1, 0])
                    nc.vector.tensor_tensor(
                        out=rsi4[:, t:t + 1], in0=cidx_f[:, t:t + 1],
                        in1=pb4[:, t:t + 1], op=fadd)

                def gather_tile(t):
                    nc.gpsimd.indirect_dma_start(
                        out=chunkcat[:, t * CH:(t + 1) * CH],
                        out_offset=None,
                        in_=pred_chunks,
                        in_offset=bass.IndirectOffsetOnAxis(
                            ap=rsi4[:, t:t + 1], axis=0))

                for t in range(N_TILES):
                    xv = xs[t][:].rearrange("p (a b) -> p a b", b=CH)
                    for r in range(rsplit):
                        nc.vector.tensor_reduce(
                            cmax4[:, t * NCHUNK + ROFF[r]:
                                  t * NCHUNK + ROFF[r + 1]],
                            xv[:, ROFF[r]:ROFF[r + 1], :],
                            axis=mybir.AxisListType.X, op=mybir.AluOpType.max)
                    if pipe and ablate != "noidx":
                        l1_tile(t)
                        gather_tile(t)

                if ablate == "noidx":
                    nc.vector.tensor_copy(acc[:, 0:1], cmax4[:, 0:1])
                    continue

                if not pipe:
                    for t in range(N_TILES):
                        nc.vector.max(
                            top8[:, t * 8:(t + 1) * 8],
                            cmax4[:, t * NCHUNK:(t + 1) * NCHUNK])
                        nc.vector.max_index(
                            cidx8[:, t * 8:(t + 1) * 8],
                            top8[:, t * 8:(t + 1) * 8],
                            cmax4[:, t * NCHUNK:(t + 1) * NCHUNK])
                    nc.vector.tensor_copy(cidx_f[:], cidx8v[:, :, 0])
                    nc.vector.tensor_tensor(
                        out=rsi4[:], in0=cidx_f[:], in1=pb4[:], op=fadd)
                    for t in range(N_TILES):
                        gather_tile(t)

                # ---- phase C: position within chunk + centers gather ----
                ctop8 = small.tile(
                    [P, N_TILES * 8], mybir.dt.float32, tag="ctop8")
                pos8 = small.tile(
                    [P, N_TILES * 8], mybir.dt.uint32, tag="pos8")
                for t in range(N_TILES):
                    nc.vector.max(
                        ctop8[:, t * 8:(t + 1) * 8],
                        chunkcat[:, t * CH:(t + 1) * CH])
                    nc.vector.max_index(
                        pos8[:, t * 8:(t + 1) * 8],
                        ctop8[:, t * 8:(t + 1) * 8],
                        chunkcat[:, t * CH:(t + 1) * CH])

                pos_f = small.tile([P, N_TILES], mybir.dt.float32, tag="pos_f")
                nc.vector.tensor_copy(
                    pos_f[:],
                    pos8[:].rearrange("p (t e) -> p t e", e=8)[:, :, 0])

                # label = 53*chunk + pos
                labi4 = small.tile([P, N_TILES], mybir.dt.int32, tag="labi4")
                nc.vector.scalar_tensor_tensor(
                    out=labi4[:], in0=cidx_f[:], scalar=float(CH),
                    in1=pos_f[:], op0=fmul, op1=fadd)

                cselcat = small.tile(
                    [P, N_TILES * FEAT_DIM], mybir.dt.float32, tag="cselcat")
                if gat1:
                    nc.gpsimd.indirect_dma_start(
                        out=cselcat[:].rearrange(
                            "p (t d) -> p t d", d=FEAT_DIM),
                        out_offset=None,
                        in_=centers.ap(),
                        in_offset=bass.IndirectOffsetOnAxis(
                            ap=labi4[:, :], axis=0))
                else:
                    for t in range(N_TILES):
                        nc.gpsimd.indirect_dma_start(
                            out=cselcat[:, t * FEAT_DIM:(t + 1) * FEAT_DIM],
                            out_offset=None,
                            in_=centers.ap(),
                            in_offset=bass.IndirectOffsetOnAxis(
                                ap=labi4[:, t:t + 1], axis=0))

                # ---- phase D: acc[p] = sum_t sum_d (f - c)^2 ----
                diff = small.tile(
                    [P, N_TILES * FEAT_DIM], mybir.dt.float32, tag="diff")
                if dsub:
                    nc.vector.tensor_tensor(
                        out=diff[:], in0=ftile[:], in1=cselcat[:],
                        op=mybir.AluOpType.subtract)
                else:
                    nc.gpsimd.tensor_sub(diff[:], ftile[:], cselcat[:])
                sq = small.tile(
                    [P, N_TILES * FEAT_DIM], mybir.dt.float32, tag="sq")
                nc.scalar.activation(
                    sq[:], diff[:], mybir.ActivationFunctionType.Square,
                    accum_out=acc[:, 0:1])

            nc.sync.dma_start(out.ap()[:, :], acc[:])

    nc.compile()
    _CACHE[key] = nc
    return nc


NPAIR = 62          # full 106-wide pair-chunks; chunk 62 is the 53-tail
PCH = 106


def _build_nc_v3(reps=1, xbufs=4, nsplit=1, dsub=1, csplit=0, dbg=0):
    """v3: fp16 cast-DMA stream + pair-chunk TT-max tree.

    Stream: SWDGE casts predicts f32->fp16 into contiguous [128, 6625]
    tiles (halves SBUF-write traffic; measured ~20% faster than the f32
    stream). Scan: 7-level tensor_tensor max tree over 62 pair-chunks of
    106 elems (212B-aligned so fp16 2x DVE mode engages; overlapped
    splits at odd levels are harmless for max) + a 1x reduce for the
    53-wide tail chunk -> 63 chunk maxes. Argmax: max8/max_index over 63
    fp16 maxes -> pair-chunk c; gather rows 2c,min(2c+1,124) of the
    [64000, 53] f32 chunk view (tail chunk fetches row 124 twice;
    first-occurrence max_index keeps pos < 53); exact f32 argmax over
    the 106 gathered values -> label = 106*c + pos. Distance phase as
    baseline (centers gather + sub + ACT Square-accumulate).
    fp16 rounding is monotone, so labels differ from f32 argmax only on
    fp16 ties at the row max (9 rows of 4096 on the seed-0 data,
    rel err 1.7e-4, gate is 2e-2).
    """
    key = ("v3", reps, xbufs, nsplit, dsub, csplit, dbg)
    if key in _CACHE:
        return _CACHE[key]

    from contextlib import ExitStack

    import concourse.bass as bass
    import concourse.tile as tile
    from concourse import bacc, mybir

    nc = bacc.Bacc(
        "TRN2",
        target_bir_lowering=False,
        debug=False,
        num_devices=N_CORES,
    )

    predicts = nc.dram_tensor(
        "predicts", [ROWS_PER_CORE, NUM_CLASSES], mybir.dt.float32,
        kind="ExternalInput",
    )
    features = nc.dram_tensor(
        "features", [ROWS_PER_CORE, FEAT_DIM], mybir.dt.float32,
        kind="ExternalInput",
    )
    centers = nc.dram_tensor(
        "centers", [NUM_CLASSES, FEAT_DIM], mybir.dt.float32,
        kind="ExternalInput",
    )
    out = nc.dram_tensor(
        "out", [P, OUT_COLS], mybir.dt.float32, kind="ExternalOutput",
    )
    if dbg:
        dbgt = nc.dram_tensor(
            "dbg", [P, 24], mybir.dt.float32, kind="ExternalOutput",
        )

    fadd = mybir.AluOpType.add
    fmul = mybir.AluOpType.mult
    fmax = mybir.AluOpType.max
    fmin = mybir.AluOpType.min
    f16 = mybir.dt.float16

    with tile.TileContext(nc) as tc:
        with ExitStack() as ctx:
            xpool = ctx.enter_context(tc.tile_pool(name="x", bufs=xbufs))
            small = ctx.enter_context(tc.tile_pool(name="small", bufs=5))
            const = ctx.enter_context(tc.tile_pool(name="const", bufs=1))
            scp = ctx.enter_context(tc.tile_pool(name="scr", bufs=1))

            # prebase[p, t] = rowid(p, t) * 125, rowid = 4p + t
            pb4_i = const.tile([P, N_TILES], mybir.dt.int32)
            nc.gpsimd.iota(
                pb4_i[:], pattern=[[NCHUNK, N_TILES]], base=0,
                channel_multiplier=N_TILES * NCHUNK)
            pb4 = const.tile([P, N_TILES], mybir.dt.float32)
            nc.vector.tensor_copy(pb4[:], pb4_i[:])
            # pmax[p, t] = prebase + 124 (last chunk row of this row)
            pmax = const.tile([P, N_TILES], mybir.dt.float32)
            nc.vector.tensor_scalar(
                out=pmax[:], in0=pb4[:], scalar1=float(NCHUNK - 1),
                scalar2=None, op0=fadd)

            acc = const.tile([P, 1], mybir.dt.float32)
            actwarm = const.tile([P, 1], mybir.dt.float32)
            nc.scalar.activation(
                actwarm[:], pb4[:, 0:1],
                mybir.ActivationFunctionType.Square)

            # pair-tree scratch (shared across tiles; DVE is in-order)
            s1 = scp.tile([P, NPAIR * 54], f16)
            s2 = scp.tile([P, NPAIR * 28], f16)
            s3 = scp.tile([P, NPAIR * 14], f16)
            s4 = scp.tile([P, NPAIR * 8], f16)
            s5 = scp.tile([P, NPAIR * 4], f16)
            s6 = scp.tile([P, NPAIR * 2], f16)
            scr = [s1[:].rearrange("p (a b) -> p a b", b=54),
                   s2[:].rearrange("p (a b) -> p a b", b=28),
                   s3[:].rearrange("p (a b) -> p a b", b=14),
                   s4[:].rearrange("p (a b) -> p a b", b=8),
                   s5[:].rearrange("p (a b) -> p a b", b=4),
                   s6[:].rearrange("p (a b) -> p a b", b=2)]

            pred_chunks = predicts.ap().rearrange("r (a b) -> (r a) b", b=CH)
            pred_v = predicts.ap().rearrange("(p t) c -> p t c", t=N_TILES)
            feat_v = features.ap().rearrange("(p t) d -> p t d", t=N_TILES)

            for _ in range(reps):
                # ---- phase A: fp16 cast stream + features ----
                xs = []
                for t in range(N_TILES):
                    x = xpool.tile([P, NUM_CLASSES], f16, tag="x", name="x")
                    if nsplit == 1:
                        nc.gpsimd.dma_start(x[:], pred_v[:, t:t + 1, :])
                    else:
                        h = (NUM_CLASSES // (2 * PCH)) * PCH  # 3180
                        nc.gpsimd.dma_start(
                            x[:, 0:h], pred_v[:, t:t + 1, 0:h])
                        nc.gpsimd.dma_start(
                            x[:, h:NUM_CLASSES],
                            pred_v[:, t:t + 1, h:NUM_CLASSES])
                    xs.append(x)
                ftile = small.tile(
                    [P, N_TILES * FEAT_DIM], mybir.dt.float32, tag="feat")
                nc.sync.dma_start(ftile[:], feat_v)

                # ---- phase B: pair-tree + L1 argmax + chunk gather ----
                cp = small.tile([P, N_TILES * 63], f16, tag="cp")
                cpv = cp[:].rearrange("p (t a) -> p t a", a=63)
                top8 = small.tile([P, N_TILES * 8], f16, tag="top8")
                cidx8 = small.tile([P, N_TILES * 8], mybir.dt.uint32,
                                   tag="cidx8")
                cidx8v = cidx8[:].rearrange("p (t e) -> p t e", e=8)
                cf = small.tile([P, N_TILES], mybir.dt.float32, tag="cf")
                tmpf = small.tile([P, N_TILES], mybir.dt.float32, tag="tmpf")
                tmpg = small.tile([P, N_TILES], mybir.dt.float32, tag="tmpg")
                rsi = small.tile([P, 2 * N_TILES], mybir.dt.int32, tag="rsi")
                rsiv = rsi[:].rearrange("p (t e) -> p t e", e=2)
                cc = small.tile([P, N_TILES * 2 * CH], mybir.dt.float32,
                                tag="cc")
                ccv = cc[:].rearrange("p (t e b) -> p t e b", e=2, b=CH)

                for t in range(N_TILES):
                    xflat = xs[t][:]
                    xp = xflat[:, 0:NPAIR * PCH].rearrange(
                        "p (a b) -> p a b", b=PCH)
                    ct = cp[:, t * 63:(t + 1) * 63]
                    nc.vector.tensor_tensor(
                        out=scr[0][:, :, :], in0=xp[:, :, 0:54],
                        in1=xp[:, :, 52:106], op=fmax)
                    nc.vector.tensor_tensor(
                        out=scr[1][:, :, :], in0=scr[0][:, :, 0:28],
                        in1=scr[0][:, :, 26:54], op=fmax)
                    nc.vector.tensor_tensor(
                        out=scr[2][:, :, :], in0=scr[1][:, :, 0:14],
                        in1=scr[1][:, :, 14:28], op=fmax)
                    nc.vector.tensor_tensor(
                        out=scr[3][:, :, :], in0=scr[2][:, :, 0:8],
                        in1=scr[2][:, :, 6:14], op=fmax)
                    nc.vector.tensor_tensor(
                        out=scr[4][:, :, :], in0=scr[3][:, :, 0:4],
                        in1=scr[3][:, :, 4:8], op=fmax)
                    nc.vector.tensor_tensor(
                        out=scr[5][:, :, :], in0=scr[4][:, :, 0:2],
                        in1=scr[4][:, :, 2:4], op=fmax)
                    nc.vector.tensor_tensor(
                        out=cpv[:, t, 0:NPAIR].rearrange(
                            "p (a b) -> p a b", b=1),
                        in0=scr[5][:, :, 0:1], in1=scr[5][:, :, 1:2],
                        op=fmax)
                    nc.vector.tensor_reduce(
                        ct[:, NPAIR:63],
                        xflat[:, NPAIR * PCH:NUM_CLASSES].rearrange(
                            "p (a b) -> p a b", a=1),
                        axis=mybir.AxisListType.X, op=fmax)

                    # L1 argmax over the 63 fp16 chunk maxes
                    nc.vector.max(top8[:, t * 8:(t + 1) * 8], ct)
                    nc.vector.max_index(
                        cidx8[:, t * 8:(t + 1) * 8],
                        top8[:, t * 8:(t + 1) * 8], ct)
                    nc.vector.tensor_copy(
                        cf[:, t:t + 1], cidx8v[:, t:t + 1, 0])
                    # rsi0 = prebase + 2c; rsi1 = min(rsi0 + 1, pmax)
                    nc.vector.scalar_tensor_tensor(
                        out=tmpf[:, t:t + 1], in0=cf[:, t:t + 1],
                        scalar=2.0, in1=pb4[:, t:t + 1],
                        op0=fmul, op1=fadd)
                    nc.vector.tensor_copy(
                        rsiv[:, t:t + 1, 0], tmpf[:, t:t + 1])
                    nc.vector.tensor_scalar(
                        out=tmpg[:, t:t + 1], in0=tmpf[:, t:t + 1],
                        scalar1=1.0, scalar2=None, op0=fadd)
                    nc.vector.tensor_tensor(
                        out=rsiv[:, t:t + 1, 1], in0=tmpg[:, t:t + 1],
                        in1=pmax[:, t:t + 1], op=fmin)
                    for e in range(2):
                        nc.gpsimd.indirect_dma_start(
                            out=ccv[:, t, e, :],
                            out_offset=None,
                            in_=pred_chunks,
                            in_offset=bass.IndirectOffsetOnAxis(
                                ap=rsi[:, 2 * t + e:2 * t + e + 1], axis=0))

                # ---- phase C: exact f32 argmax within 106-wide chunk ----
                ctop8 = small.tile([P, N_TILES * 8], mybir.dt.float32,
                                   tag="ctop8")
                pos8 = small.tile([P, N_TILES * 8], mybir.dt.uint32,
                                  tag="pos8")
                pos_f = small.tile([P, N_TILES], mybir.dt.float32,
                                   tag="pos_f")
                labi4 = small.tile([P, N_TILES], mybir.dt.int32, tag="labi4")
                for t in range(N_TILES):
                    nc.vector.max(
                        ctop8[:, t * 8:(t + 1) * 8],
                        cc[:, t * 2 * CH:(t + 1) * 2 * CH])
                    nc.vector.max_index(
                        pos8[:, t * 8:(t + 1) * 8],
                        ctop8[:, t * 8:(t + 1) * 8],
                        cc[:, t * 2 * CH:(t + 1) * 2 * CH])
                nc.vector.tensor_copy(
                    pos_f[:],
                    pos8[:].rearrange("p (t e) -> p t e", e=8)[:, :, 0])
                # label = 106*c + pos
                nc.vector.scalar_tensor_tensor(
                    out=labi4[:], in0=cf[:], scalar=float(PCH),
                    in1=pos_f[:], op0=fmul, op1=fadd)

                cselcat = small.tile(
                    [P, N_TILES * FEAT_DIM], mybir.dt.float32, tag="cselcat")
                for t in range(N_TILES):
                    nc.gpsimd.indirect_dma_start(
                        out=cselcat[:, t * FEAT_DIM:(t + 1) * FEAT_DIM],
                        out_offset=None,
                        in_=centers.ap(),
                        in_offset=bass.IndirectOffsetOnAxis(
                            ap=labi4[:, t:t + 1], axis=0))

                # ---- phase D: acc[p] = sum_t sum_d (f - c)^2 ----
                diff = small.tile(
                    [P, N_TILES * FEAT_DIM], mybir.dt.float32, tag="diff")
                if dsub:
                    nc.vector.tensor_tensor(
                        out=diff[:], in0=ftile[:], in1=cselcat[:],
                        op=mybir.AluOpType.subtract)
                else:
                    nc.gpsimd.tensor_sub(diff[:], ftile[:], cselcat[:])
                sq = small.tile(
                    [P, N_TILES * FEAT_DIM], mybir.dt.float32, tag="sq")
                nc.scalar.activation(
                    sq[:], diff[:], mybir.ActivationFunctionType.Square,
                    accum_out=acc[:, 0:1])

            if dbg:
                dbuf = small.tile([P, 24], mybir.dt.float32, tag="dbg")
                nc.vector.tensor_copy(dbuf[:, 0:4], cf[:])
                nc.vector.tensor_copy(dbuf[:, 4:8], pos_f[:])
                nc.vector.tensor_copy(dbuf[:, 8:12], labi4[:])
                nc.vector.tensor_copy(dbuf[:, 12:20], rsi[:])
                nc.vector.tensor_copy(dbuf[:, 20:24],
                                      cp[:].rearrange(
                                          "p (t a) -> p t a", a=63)[:, :, 0])
                nc.sync.dma_start(dbgt.ap()[:, :], dbuf[:])
            nc.sync.dma_start(out.ap()[:, :], acc[:])

    nc.compile()
    _CACHE[key] = nc
    return nc


def kernel(features, predicts, centers):
    from concourse.bass_utils import run_bass_kernel_spmd

    nc = _build_nc()

    feats = np.ascontiguousarray(
        np.asarray(features, dtype=np.float32).reshape(N_ROWS, FEAT_DIM))
    preds = np.ascontiguousarray(
        np.asarray(predicts, dtype=np.float32).reshape(N_ROWS, NUM_CLASSES))
    cents = np.ascontiguousarray(np.asarray(centers, dtype=np.float32))

    in_maps = []
    for m in range(N_CORES):
        s = slice(m * ROWS_PER_CORE, (m + 1) * ROWS_PER_CORE)
        in_maps.append({
            "predicts": np.ascontiguousarray(preds[s]),
            "features": np.ascontiguousarray(feats[s]),
            "centers": cents,
        })

    res = run_bass_kernel_spmd(nc, in_maps, core_ids=list(range(N_CORES)))

    d = np.concatenate([r["out"].reshape(-1) for r in res.results])
    d = np.clip(d.astype(np.float64), 1e-12, 1e12)
    total = d.sum() + (N_ROWS * NUM_CLASSES - N_ROWS) * 1e-12
    return np.asarray(total / N_ROWS, dtype=np.float32)



# revision 13
# speedup vs baseline: 1.1448x; 1.1448x over previous
"""CenterLoss Trainium2 kernel.

reference semantics:
    feats  = features.reshape(4096, 96)
    label  = argmax(predicts.reshape(4096, 6625), axis=1)   # first occurrence
    d[n]   = ||feats[n] - centers[label[n]]||^2
    loss   = (sum_n clip(d[n], 1e-12, 1e12) + (4096*6625-4096)*1e-12) / 4096

Sharding: data-parallel over the flattened 4096-row batch axis, 512 rows per
core across 8 cores; centers replicated. Each core returns 128 per-partition
distance sums; the host does the final (tiny) reduction ("all-reduce").

Per-core pipeline (phases batched over the 4 row-tiles of 128 rows each so
engines never stall on each other's in-order queues; partition p holds rows
4p..4p+3 so the features DMA is one contiguous 1536B line per partition):
  A. stream predicts tiles [128, 6625] HBM->SBUF in two half-tile DMAs
     (~1.7 MB each, 13.3KB/partition descriptors; the memory-bound part,
     ~13.6 MB/core) + one features DMA
  B. per-half max-reduce over [128, chunks, 53] views -> cmax [128, 4*125];
     per-tile DVE max8 -> row max top8, max_index -> first-occurrence
     chunk id (uint32)
  C. indirect-DMA gather of each row's winning 53-wide chunk (212B/row,
     ~108KB/core extra HBM vs 543KB for 265-wide chunks); max8+max_index on
     the gathered chunk -> position; label = 53*chunk + pos; indirect-DMA
     gather of centers[label]
  D. acc[p] = sum over tiles/dims of (f - c)^2 via one gpsimd subtract +
     one ACT Square with accumulate
max8/max_index tie-breaking is first-occurrence, matching jnp.argmax
bit-exactly (f32 compares are exact).
"""

import numpy as np

NUM_CLASSES = 6625
FEAT_DIM = 96
N_ROWS = 4096           # B*T = 64*64
N_CORES = 8
ROWS_PER_CORE = N_ROWS // N_CORES   # 512
P = 128                 # partitions
N_TILES = ROWS_PER_CORE // P        # 4 row-tiles per core
CH = 53                 # chunk size (6625 = 125 * 53)
NCHUNK = NUM_CLASSES // CH          # 125
OUT_COLS = 1            # per-partition distance sums

_CACHE = {}
_DEFAULT_VER = 3


def _splits(n, k):
    """k split sizes of n, near-equal, larger first."""
    q, r = divmod(n, k)
    return [q + 1] * r + [q] * (k - r)


def _build_nc(reps=1, ver=None, **kw):
    """Dispatch: ver=3 (default) = fp16 pair-tree kernel; ver=0 = baseline."""
    ver = _DEFAULT_VER if ver is None else ver
    if ver == 3:
        return _build_nc_v3(reps=reps, **kw)
    return _build_nc_base(reps=reps, **kw)


def _build_nc_base(reps=1, ablate="full", nsplit=2, rsplit=None, dual=0,
                   dsub=0, ilv=1, xbufs=4, gat1=0, seng=0, pipe=0):
    if rsplit is None:
        rsplit = nsplit
    assert nsplit % rsplit == 0
    key = ("nc", reps, ablate, nsplit, rsplit, dual, dsub, ilv, xbufs, gat1,
           seng, pipe)
    if key in _CACHE:
        return _CACHE[key]

    from contextlib import ExitStack

    import concourse.bass as bass
    import concourse.tile as tile
    from concourse import bacc, mybir

    nc = bacc.Bacc(
        "TRN2",
        target_bir_lowering=False,
        debug=False,
        num_devices=N_CORES,
    )

    predicts = nc.dram_tensor(
        "predicts", [ROWS_PER_CORE, NUM_CLASSES], mybir.dt.float32,
        kind="ExternalInput",
    )
    features = nc.dram_tensor(
        "features", [ROWS_PER_CORE, FEAT_DIM], mybir.dt.float32,
        kind="ExternalInput",
    )
    centers = nc.dram_tensor(
        "centers", [NUM_CLASSES, FEAT_DIM], mybir.dt.float32,
        kind="ExternalInput",
    )
    out = nc.dram_tensor(
        "out", [P, OUT_COLS], mybir.dt.float32, kind="ExternalOutput",
    )

    fadd = mybir.AluOpType.add
    fmul = mybir.AluOpType.mult

    with tile.TileContext(nc) as tc:
        with ExitStack() as ctx:
            xpool = ctx.enter_context(tc.tile_pool(name="x", bufs=xbufs))
            small = ctx.enter_context(tc.tile_pool(name="small", bufs=5))
            const = ctx.enter_context(tc.tile_pool(name="const", bufs=1))

            # prebase[p, t] = rowid(p, t) * 125 : global chunk-row id base
            # rowid = t*128 + p normally; 4p + t with interleaved mapping
            pb4_i = const.tile([P, N_TILES], mybir.dt.int32)
            if ilv:
                nc.gpsimd.iota(
                    pb4_i[:], pattern=[[NCHUNK, N_TILES]], base=0,
                    channel_multiplier=N_TILES * NCHUNK)
            else:
                nc.gpsimd.iota(
                    pb4_i[:], pattern=[[P * NCHUNK, N_TILES]], base=0,
                    channel_multiplier=NCHUNK)
            pb4 = const.tile([P, N_TILES], mybir.dt.float32)
            nc.vector.tensor_copy(pb4[:], pb4_i[:])

            # per-partition distance accumulator (summed over row tiles)
            acc = const.tile([P, 1], mybir.dt.float32)

            # warm the ACT Square table set while DMAs stream
            actwarm = const.tile([P, 1], mybir.dt.float32)
            nc.scalar.activation(
                actwarm[:], pb4[:, 0:1],
                mybir.ActivationFunctionType.Square)

            # predicts viewed as rows of 53 elements: [512*125, 53]
            pred_chunks = predicts.ap().rearrange("r (a b) -> (r a) b", b=CH)
            if ilv:
                # partition p holds rows 4p..4p+3; tile t = rows {4p+t}
                pred_v = predicts.ap().rearrange(
                    "(p t) c -> p t c", t=N_TILES)
                feat_v = features.ap().rearrange(
                    "(p t) d -> p t d", t=N_TILES)
            else:
                pred_v = None
                feat_v = features.ap().rearrange("(t p) d -> p t d", p=P)

            # chunk-count split per partial DMA; reduces cover rsplit
            # groups of nsplit//rsplit DMA splits each
            QSPLIT = _splits(NCHUNK, nsplit)
            QOFF = [0]
            for q in QSPLIT:
                QOFF.append(QOFF[-1] + q)
            step = nsplit // rsplit
            ROFF = [QOFF[i * step] for i in range(rsplit)] + [NCHUNK]

            for _ in range(reps):
                # ---- phase A: stream predicts + features ----
                xs = []
                for t in range(N_TILES):
                    x = xpool.tile([P, NUM_CLASSES], mybir.dt.float32, tag="x")
                    for q in range(nsplit):
                        c0, c1 = QOFF[q] * CH, QOFF[q + 1] * CH
                        if seng:
                            eng = nc.gpsimd
                        else:
                            eng = nc.scalar if (dual and q % 2) else nc.sync
                        if ilv:
                            src = pred_v[:, t:t + 1, c0:c1]
                        else:
                            src = predicts.ap()[t * P:(t + 1) * P, c0:c1]
                        eng.dma_start(x[:, c0:c1], src)
                    xs.append(x)
                ftile = small.tile(
                    [P, N_TILES * FEAT_DIM], mybir.dt.float32, tag="feat")
                nc.sync.dma_start(ftile[:], feat_v)

                if ablate == "dma":
                    for t in range(N_TILES):
                        xv = xs[t][:].rearrange("p (a b) -> p a b", b=CH)
                        nc.vector.tensor_reduce(
                            acc[:, 0:1], xv[:, 0:1, :],
                            axis=mybir.AxisListType.XY, op=mybir.AluOpType.max)
                        nc.vector.tensor_reduce(
                            acc[:, 0:1], xv[:, NCHUNK - 1:NCHUNK, :],
                            axis=mybir.AxisListType.XY, op=mybir.AluOpType.max)
                    continue

                # ---- phase B: chunk maxes + per-tile top8/argmax chunk ----
                # pipe mode: each tile's L1 argmax + chunk gather issue right
                # after that tile's reduces, so the SWDGE gather latency
                # hides under the next tile's reduce work instead of
                # stalling the in-order DVE queue before phase C.
                cmax4 = small.tile(
                    [P, N_TILES * NCHUNK], mybir.dt.float32, tag="cmax4")
                top8 = small.tile(
                    [P, N_TILES * 8], mybir.dt.float32, tag="top8")
                cidx8 = small.tile(
                    [P, N_TILES * 8], mybir.dt.uint32, tag="cidx8")
                cidx_f = small.tile([P, N_TILES], mybir.dt.float32,
                                    tag="cidx_f")
                rsi4 = small.tile([P, N_TILES], mybir.dt.int32, tag="rsi4")
                chunkcat = small.tile(
                    [P, N_TILES * CH], mybir.dt.float32, tag="chunkcat")
                cidx8v = cidx8[:].rearrange("p (t e) -> p t e", e=8)

                def l1_tile(t):
                    nc.vector.max(
                        top8[:, t * 8:(t + 1) * 8],
                        cmax4[:, t * NCHUNK:(t + 1) * NCHUNK])
                    nc.vector.max_index(
                        cidx8[:, t * 8:(t + 1) * 8],
                        top8[:, t * 8:(t + 1) * 8],
                        cmax4[:, t * NCHUNK:(t + 1) * NCHUNK])
                    nc.vector.tensor_copy(
                        cidx_f[:, t:t + 1], cidx8v[:, t:t + 1, 0])
                    nc.vector.tensor_tensor(
                        out=rsi4[:, t:t + 1], in0=cidx_f[:, t:t + 1],
                        in1=pb4[:, t:t + 1], op=fadd)

                def gather_tile(t):
                    nc.gpsimd.indirect_dma_start(
                        out=chunkcat[:, t * CH:(t + 1) * CH],
                        out_offset=None,
                        in_=pred_chunks,
                        in_offset=bass.IndirectOffsetOnAxis(
                            ap=rsi4[:, t:t + 1], axis=0))

                for t in range(N_TILES):
                    xv = xs[t][:].rearrange("p (a b) -> p a b", b=CH)
                    for r in range(rsplit):
                        nc.vector.tensor_reduce(
                            cmax4[:, t * NCHUNK + ROFF[r]:
                                  t * NCHUNK + ROFF[r + 1]],
                            xv[:, ROFF[r]:ROFF[r + 1], :],
                            axis=mybir.AxisListType.X, op=mybir.AluOpType.max)
                    if pipe and ablate != "noidx":
                        l1_tile(t)
                        gather_tile(t)

                if ablate == "noidx":
                    nc.vector.tensor_copy(acc[:, 0:1], cmax4[:, 0:1])
                    continue

                if not pipe:
                    for t in range(N_TILES):
                        nc.vector.max(
                            top8[:, t * 8:(t + 1) * 8],
                            cmax4[:, t * NCHUNK:(t + 1) * NCHUNK])
                        nc.vector.max_index(
                            cidx8[:, t * 8:(t + 1) * 8],
                            top8[:, t * 8:(t + 1) * 8],
                            cmax4[:, t * NCHUNK:(t + 1) * NCHUNK])
                    nc.vector.tensor_copy(cidx_f[:], cidx8v[:, :, 0])
                    nc.vector.tensor_tensor(
                        out=rsi4[:], in0=cidx_f[:], in1=pb4[:], op=fadd)
                    for t in range(N_TILES):
                        gather_tile(t)

                # ---- phase C: position within chunk + centers gather ----
                ctop8 = small.tile(
                    [P, N_TILES * 8], mybir.dt.float32, tag="ctop8")
                pos8 = small.tile(
                    [P, N_TILES * 8], mybir.dt.uint32, tag="pos8")
                for t in range(N_TILES):
                    nc.vector.max(
                        ctop8[:, t * 8:(t + 1) * 8],
                        chunkcat[:, t * CH:(t + 1) * CH])
                    nc.vector.max_index(
                        pos8[:, t * 8:(t + 1) * 8],
                        ctop8[:, t * 8:(t + 1) * 8],
                        chunkcat[:, t * CH:(t + 1) * CH])

                pos_f = small.tile([P, N_TILES], mybir.dt.float32, tag="pos_f")
                nc.vector.tensor_copy(
                    pos_f[:],
                    pos8[:].rearrange("p (t e) -> p t e", e=8)[:, :, 0])

                # label = 53*chunk + pos
                labi4 = small.tile([P, N_TILES], mybir.dt.int32, tag="labi4")
                nc.vector.scalar_tensor_tensor(
                    out=labi4[:], in0=cidx_f[:], scalar=float(CH),
                    in1=pos_f[:], op0=fmul, op1=fadd)

                cselcat = small.tile(
                    [P, N_TILES * FEAT_DIM], mybir.dt.float32, tag="cselcat")
                if gat1:
                    nc.gpsimd.indirect_dma_start(
                        out=cselcat[:].rearrange(
                            "p (t d) -> p t d", d=FEAT_DIM),
                        out_offset=None,
                        in_=centers.ap(),
                        in_offset=bass.IndirectOffsetOnAxis(
                            ap=labi4[:, :], axis=0))
                else:
                    for t in range(N_TILES):
                        nc.gpsimd.indirect_dma_start(
                            out=cselcat[:, t * FEAT_DIM:(t + 1) * FEAT_DIM],
                            out_offset=None,
                            in_=centers.ap(),
                            in_offset=bass.IndirectOffsetOnAxis(
                                ap=labi4[:, t:t + 1], axis=0))

                # ---- phase D: acc[p] = sum_t sum_d (f - c)^2 ----
                diff = small.tile(
                    [P, N_TILES * FEAT_DIM], mybir.dt.float32, tag="diff")
                if dsub:
                    nc.vector.tensor_tensor(
                        out=diff[:], in0=ftile[:], in1=cselcat[:],
                        op=mybir.AluOpType.subtract)
                else:
                    nc.gpsimd.tensor_sub(diff[:], ftile[:], cselcat[:])
                sq = small.tile(
                    [P, N_TILES * FEAT_DIM], mybir.dt.float32, tag="sq")
                nc.scalar.activation(
                    sq[:], diff[:], mybir.ActivationFunctionType.Square,
                    accum_out=acc[:, 0:1])

            nc.sync.dma_start(out.ap()[:, :], acc[:])

    nc.compile()
    _CACHE[key] = nc
    return nc


NPAIR = 62          # full 106-wide pair-chunks; chunk 62 is the 53-tail
PCH = 106


def _build_nc_v3(reps=1, xbufs=4, nsplit=1, dsub=1, csplit=0, dbg=0):
    """v3: fp16 cast-DMA stream + pair-chunk TT-max tree.

    Stream: SWDGE casts predicts f32->fp16 into contiguous [128, 6625]
    tiles (halves SBUF-write traffic; measured ~20% faster than the f32
    stream). Scan: 7-level tensor_tensor max tree over 62 pair-chunks of
    106 elems (212B-aligned so fp16 2x DVE mode engages; overlapped
    splits at odd levels are harmless for max) + a 1x reduce for the
    53-wide tail chunk -> 63 chunk maxes. Argmax: max8/max_index over 63
    fp16 maxes -> pair-chunk c; gather rows 2c,min(2c+1,124) of the
    [64000, 53] f32 chunk view (tail chunk fetches row 124 twice;
    first-occurrence max_index keeps pos < 53); exact f32 argmax over
    the 106 gathered values -> label = 106*c + pos. Distance phase as
    baseline (centers gather + sub + ACT Square-accumulate).
    fp16 rounding is monotone, so labels differ from f32 argmax only on
    fp16 ties at the row max (9 rows of 4096 on the seed-0 data,
    rel err 1.7e-4, gate is 2e-2).
    """
    key = ("v3", reps, xbufs, nsplit, dsub, csplit, dbg)
    if key in _CACHE:
        return _CACHE[key]

    from contextlib import ExitStack

    import concourse.bass as bass
    import concourse.tile as tile
    from concourse import bacc, mybir

    nc = bacc.Bacc(
        "TRN2",
        target_bir_lowering=False,
        debug=False,
        num_devices=N_CORES,
    )

    predicts = nc.dram_tensor(
        "predicts", [ROWS_PER_CORE, NUM_CLASSES], mybir.dt.float32,
        kind="ExternalInput",
    )
    features = nc.dram_tensor(
        "features", [ROWS_PER_CORE, FEAT_DIM], mybir.dt.float32,
        kind="ExternalInput",
    )
    centers = nc.dram_tensor(
        "centers", [NUM_CLASSES, FEAT_DIM], mybir.dt.float32,
        kind="ExternalInput",
    )
    out = nc.dram_tensor(
        "out", [P, OUT_COLS], mybir.dt.float32, kind="ExternalOutput",
    )
    if dbg:
        dbgt = nc.dram_tensor(
            "dbg", [P, 24], mybir.dt.float32, kind="ExternalOutput",
        )

    fadd = mybir.AluOpType.add
    fmul = mybir.AluOpType.mult
    fmax = mybir.AluOpType.max
    fmin = mybir.AluOpType.min
    f16 = mybir.dt.float16

    with tile.TileContext(nc) as tc:
        with ExitStack() as ctx:
            xpool = ctx.enter_context(tc.tile_pool(name="x", bufs=xbufs))
            small = ctx.enter_context(tc.tile_pool(name="small", bufs=5))
            const = ctx.enter_context(tc.tile_pool(name="const", bufs=1))
            scp = ctx.enter_context(tc.tile_pool(name="scr", bufs=1))

            # prebase[p, t] = rowid(p, t) * 125, rowid = 4p + t
            pb4_i = const.tile([P, N_TILES], mybir.dt.int32)
            nc.gpsimd.iota(
                pb4_i[:], pattern=[[NCHUNK, N_TILES]], base=0,
                channel_multiplier=N_TILES * NCHUNK)
            pb4 = const.tile([P, N_TILES], mybir.dt.float32)
            nc.vector.tensor_copy(pb4[:], pb4_i[:])

            acc = const.tile([P, 1], mybir.dt.float32)
            actwarm = const.tile([P, 1], mybir.dt.float32)
            nc.scalar.activation(
                actwarm[:], pb4[:, 0:1],
                mybir.ActivationFunctionType.Square)

            # pair-tree scratch (shared across tiles; DVE is in-order)
            s1 = scp.tile([P, NPAIR * 54], f16)
            s2 = scp.tile([P, NPAIR * 28], f16)
            s3 = scp.tile([P, NPAIR * 14], f16)
            s4 = scp.tile([P, NPAIR * 8], f16)
            s5 = scp.tile([P, NPAIR * 4], f16)
            s6 = scp.tile([P, NPAIR * 2], f16)
            scr = [s1[:].rearrange("p (a b) -> p a b", b=54),
                   s2[:].rearrange("p (a b) -> p a b", b=28),
                   s3[:].rearrange("p (a b) -> p a b", b=14),
                   s4[:].rearrange("p (a b) -> p a b", b=8),
                   s5[:].rearrange("p (a b) -> p a b", b=4),
                   s6[:].rearrange("p (a b) -> p a b", b=2)]

            pred_chunks = predicts.ap().rearrange("r (a b) -> (r a) b", b=CH)
            pred_v = predicts.ap().rearrange("(p t) c -> p t c", t=N_TILES)
            feat_v = features.ap().rearrange("(p t) d -> p t d", t=N_TILES)

            # ---- software-pipelined stages ----
            # body i emits: gpsimd [stream(i), window-gathers(i-1),
            # centers-gather(i-2)], DVE [phaseC(i-1), phaseD(i-2),
            # trees+L1+idx(i)]. All gpsimd ops then wait only on
            # long-ready data, so the in-order SWDGE queue never
            # head-of-line-blocks the next rep's stream emission.

            def stage_a():
                """Stream: fp16 cast DMAs + features."""
                xs = []
                for t in range(N_TILES):
                    x = xpool.tile([P, NUM_CLASSES], f16, tag="x", name="x")
                    nc.gpsimd.dma_start(x[:], pred_v[:, t:t + 1, :])
                    xs.append(x)
                ftile = small.tile(
                    [P, N_TILES * FEAT_DIM], mybir.dt.float32, tag="feat",
                    name="ftile")
                nc.sync.dma_start(ftile[:], feat_v)
                return {"xs": xs, "ftile": ftile}

            def stage_b(st):
                """DVE: pair-trees, L1 argmax, window index i0=min(2c,123)."""
                cp = small.tile([P, N_TILES * 63], f16, tag="cp", name="cp")
                cpv = cp[:].rearrange("p (t a) -> p t a", a=63)
                top8 = small.tile([P, N_TILES * 8], f16, tag="top8",
                                  name="top8")
                cidx8 = small.tile([P, N_TILES * 8], mybir.dt.uint32,
                                   tag="cidx8", name="cidx8")
                cidx8v = cidx8[:].rearrange("p (t e) -> p t e", e=8)
                cf = small.tile([P, N_TILES], mybir.dt.float32, tag="cf",
                                name="cf")
                i0f = small.tile([P, N_TILES], mybir.dt.float32, tag="i0f",
                                 name="i0f")
                rsi = small.tile([P, N_TILES], mybir.dt.int32, tag="rsi",
                                 name="rsi")
                for t in range(N_TILES):
                    xflat = st["xs"][t][:]
                    xp = xflat[:, 0:NPAIR * PCH].rearrange(
                        "p (a b) -> p a b", b=PCH)
                    ct = cp[:, t * 63:(t + 1) * 63]
                    nc.vector.tensor_tensor(
                        out=scr[0][:, :, :], in0=xp[:, :, 0:54],
                        in1=xp[:, :, 52:106], op=fmax)
                    nc.vector.tensor_tensor(
                        out=scr[1][:, :, :], in0=scr[0][:, :, 0:28],
                        in1=scr[0][:, :, 26:54], op=fmax)
                    nc.vector.tensor_tensor(
                        out=scr[2][:, :, :], in0=scr[1][:, :, 0:14],
                        in1=scr[1][:, :, 14:28], op=fmax)
                    nc.vector.tensor_tensor(
                        out=scr[3][:, :, :], in0=scr[2][:, :, 0:8],
                        in1=scr[2][:, :, 6:14], op=fmax)
                    nc.vector.tensor_tensor(
                        out=scr[4][:, :, :], in0=scr[3][:, :, 0:4],
                        in1=scr[3][:, :, 4:8], op=fmax)
                    nc.vector.tensor_tensor(
                        out=scr[5][:, :, :], in0=scr[4][:, :, 0:2],
                        in1=scr[4][:, :, 2:4], op=fmax)
                    nc.vector.tensor_tensor(
                        out=cpv[:, t, 0:NPAIR].rearrange(
                            "p (a b) -> p a b", b=1),
                        in0=scr[5][:, :, 0:1], in1=scr[5][:, :, 1:2],
                        op=fmax)
                    nc.vector.tensor_reduce(
                        ct[:, NPAIR:63],
                        xflat[:, NPAIR * PCH:NUM_CLASSES].rearrange(
                            "p (a b) -> p a b", a=1),
                        axis=mybir.AxisListType.X, op=fmax)
                    # L1 argmax over the 63 fp16 chunk maxes
                    nc.vector.max(top8[:, t * 8:(t + 1) * 8], ct)
                    nc.vector.max_index(
                        cidx8[:, t * 8:(t + 1) * 8],
                        top8[:, t * 8:(t + 1) * 8], ct)
                    nc.vector.tensor_copy(
                        cf[:, t:t + 1], cidx8v[:, t:t + 1, 0])
                    # i0 = min(2c, 123); rsi = prebase + i0 (53-row units)
                    nc.vector.tensor_scalar(
                        out=i0f[:, t:t + 1], in0=cf[:, t:t + 1],
                        scalar1=2.0, scalar2=float(2 * NPAIR - 1),
                        op0=fmul, op1=fmin)
                    nc.vector.tensor_tensor(
                        out=rsi[:, t:t + 1], in0=i0f[:, t:t + 1],
                        in1=pb4[:, t:t + 1], op=fadd)
                st.update(cp=cp, cf=cf, i0f=i0f, rsi=rsi)

            def gathers_b(st):
                """gpsimd: one 106-wide window gather per tile."""
                cc = small.tile([P, N_TILES * PCH], mybir.dt.float32,
                                tag="cc", name="cc")
                ccv = cc[:].rearrange("p (t e b) -> p t e b", e=2, b=CH)
                for t in range(N_TILES):
                    for e in range(2):
                        nc.gpsimd.indirect_dma_start(
                            out=ccv[:, t, e, :],
                            out_offset=None,
                            in_=pred_chunks,
                            in_offset=bass.IndirectOffsetOnAxis(
                                ap=st["rsi"][:, t:t + 1], axis=0),
                            element_offset=e * CH)
                st.update(cc=cc)

            def stage_c(st):
                """DVE: exact f32 argmax in the 106 window + label."""
                cc = st["cc"]
                ctop8 = small.tile([P, N_TILES * 8], mybir.dt.float32,
                                   tag="ctop8", name="ctop8")
                pos8 = small.tile([P, N_TILES * 8], mybir.dt.uint32,
                                  tag="pos8", name="pos8")
                pos_f = small.tile([P, N_TILES], mybir.dt.float32,
                                   tag="pos_f", name="pos_f")
                labi4 = small.tile([P, N_TILES], mybir.dt.int32,
                                   tag="labi4", name="labi4")
                for t in range(N_TILES):
                    nc.vector.max(
                        ctop8[:, t * 8:(t + 1) * 8],
                        cc[:, t * PCH:(t + 1) * PCH])
                    nc.vector.max_index(
                        pos8[:, t * 8:(t + 1) * 8],
                        ctop8[:, t * 8:(t + 1) * 8],
                        cc[:, t * PCH:(t + 1) * PCH])
                nc.vector.tensor_copy(
                    pos_f[:],
                    pos8[:].rearrange("p (t e) -> p t e", e=8)[:, :, 0])
                # label = 53*i0 + pos
                nc.vector.scalar_tensor_tensor(
                    out=labi4[:], in0=st["i0f"], scalar=float(CH),
                    in1=pos_f[:], op0=fmul, op1=fadd)
                st.update(pos_f=pos_f, labi4=labi4)

            def gather_c(st):
                """gpsimd: centers gathers (one per tile)."""
                cselcat = small.tile(
                    [P, N_TILES * FEAT_DIM], mybir.dt.float32,
                    tag="cselcat", name="cselcat")
                for t in range(N_TILES):
                    nc.gpsimd.indirect_dma_start(
                        out=cselcat[:, t * FEAT_DIM:(t + 1) * FEAT_DIM],
                        out_offset=None,
                        in_=centers.ap(),
                        in_offset=bass.IndirectOffsetOnAxis(
                            ap=st["labi4"][:, t:t + 1], axis=0))
                st.update(cselcat=cselcat)

            def stage_d(st):
                """DVE sub + ACT Square-accumulate."""
                diff = small.tile(
                    [P, N_TILES * FEAT_DIM], mybir.dt.float32, tag="diff",
                    name="diff")
                if dsub:
                    nc.vector.tensor_tensor(
                        out=diff[:], in0=st["ftile"][:], in1=st["cselcat"][:],
                        op=mybir.AluOpType.subtract)
                else:
                    nc.gpsimd.tensor_sub(
                        diff[:], st["ftile"][:], st["cselcat"][:])
                sq = small.tile(
                    [P, N_TILES * FEAT_DIM], mybir.dt.float32, tag="sq",
                    name="sq")
                nc.scalar.activation(
                    sq[:], diff[:], mybir.ActivationFunctionType.Square,
                    accum_out=acc[:, 0:1])

            hist = []
            for i in range(reps):
                st = stage_a()
                if i >= 1:
                    gathers_b(hist[-1])
                if i >= 2:
                    gather_c(hist[-2])
                if i >= 1:
                    stage_c(hist[-1])
                if i >= 2:
                    stage_d(hist[-2])
                stage_b(st)
                hist.append(st)
                if len(hist) > 3:
                    hist.pop(0)

            # flush the pipeline tail
            gathers_b(hist[-1])
            if reps >= 2:
                gather_c(hist[-2])
            stage_c(hist[-1])
            if reps >= 2:
                stage_d(hist[-2])
            gather_c(hist[-1])
            stage_d(hist[-1])

            if dbg:
                st = hist[-1]
                dbuf = small.tile([P, 24], mybir.dt.float32, tag="dbg")
                nc.vector.tensor_copy(dbuf[:, 0:4], st["cf"][:])
                nc.vector.tensor_copy(dbuf[:, 4:8], st["pos_f"][:])
                nc.vector.tensor_copy(dbuf[:, 8:12], st["labi4"][:])
                nc.vector.tensor_copy(dbuf[:, 12:16], st["rsi"][:])
                nc.vector.tensor_copy(dbuf[:, 16:20], st["i0f"][:])
                nc.vector.tensor_copy(dbuf[:, 20:24],
                                      st["cp"][:].rearrange(
                                          "p (t a) -> p t a", a=63)[:, :, 0])
                nc.sync.dma_start(dbgt.ap()[:, :], dbuf[:])
            nc.sync.dma_start(out.ap()[:, :], acc[:])

    nc.compile()
    _CACHE[key] = nc
    return nc


def kernel(features, predicts, centers):
    from concourse.bass_utils import run_bass_kernel_spmd

    nc = _build_nc()

    feats = np.ascontiguousarray(
        np.asarray(features, dtype=np.float32).reshape(N_ROWS, FEAT_DIM))
    preds = np.ascontiguousarray(
        np.asarray(predicts, dtype=np.float32).reshape(N_ROWS, NUM_CLASSES))
    cents = np.ascontiguousarray(np.asarray(centers, dtype=np.float32))

    in_maps = []
    for m in range(N_CORES):
        s = slice(m * ROWS_PER_CORE, (m + 1) * ROWS_PER_CORE)
        in_maps.append({
            "predicts": np.ascontiguousarray(preds[s]),
            "features": np.ascontiguousarray(feats[s]),
            "centers": cents,
        })

    res = run_bass_kernel_spmd(nc, in_maps, core_ids=list(range(N_CORES)))

    d = np.concatenate([r["out"].reshape(-1) for r in res.results])
    d = np.clip(d.astype(np.float64), 1e-12, 1e12)
    total = d.sum() + (N_ROWS * NUM_CLASSES - N_ROWS) * 1e-12
    return np.asarray(total / N_ROWS, dtype=np.float32)

